# revision 22
# baseline (speedup 1.0000x reference)
"""Trainium2 kernel for AdaptiveAttention (QKV projection + causal
sliding-window attention, span=128) on 8 NeuronCores.

Sharding: sequence-parallel with a 1-block halo — 8 shards of
(batch b, sequence half h): each core owns 2048 query tokens and receives
128 halo tokens of x so the previous block's K/V context is local.
No collectives are needed (window attention is local).

Algorithmic fusion (the big win vs a direct QKV translation): softmax is
invariant to adding a per-query constant to the scores, so

    S = (x Wq^T + bq) (x Wk^T + bk)^T
      ~ x (Wq^T Wk) x^T + (bq Wk) x^T        (per-query terms dropped)

With A = scale * Wq^T Wk and w = scale * bq Wk precomputed on the HOST,
the whole K projection disappears from the device: the kernel computes
Z^T = A^T-contract(x) + w once per query chunk (exactly the cost of the
old Q projection) and scores contract Z directly against raw x tiles
(which are already resident in d-major layout for the projections).
Similarly bv is not added to V: sum_j softmax_ij (v_j + bv) = (...) + bv,
so bv is folded into the final normalize (one fused DVE op).

Per-core layout strategy:
  - All matmuls run in f16 (inputs rounded on host; the 2^-11 rounding is
    comparable to the f32r PE path's own rounding) at full PE rate.
  - x is passed pre-transposed (d-major) so the Z projection, V projection
    and the score matmuls all use resident tiles with no on-chip
    transposes:
      Z^T[d',q] = sum_d A[d,d'] * xT[d,q]     (lhsT=A tile,  rhs=xT)
      V[t,e]    = sum_d xT[d,t] * WvT[d,e]    (lhsT=xT tile, rhs=WvT)
      sT[k,q]   = sum_d xT[d,k] * Z^T[d,q]    (lhsT=xT tile, rhs=Z^T)
  - Scores are computed per 128-query half as two [128,128] blocks (only
    the blocks the window can touch). The causal/window mask is ADDITIVE
    (-1e4) and applied by the PE itself: each score accumulation chain is
    SEEDED with an identity-lhsT matmul of the mask tile (128 rows, ~2%
    of chunk PE time), so no non-PE engine sits between the score chains
    and the exp. exp runs on ACT straight from PSUM with a folded -4
    bias (cancels in softmax, keeps everything in f16 range), then
      o[q,e] = sum_k wT[k,q] * V[k,e]
    with the softmax denominator from a ones-column matmul into the score
    PSUM tile's tail; the final normalize is a per-partition scalar
    multiply fused with the +bv add and the PSUM->SBUF f16 cast.
  - Engine p-state discipline (the big scheduling lesson: an engine idle
    for >~5us drops to a low p-state and its next op runs ~10x slow, and
    any PE gap also resets the PE clock to 1.2GHz for ~3us): PSUM->SBUF
    copies alternate between DVE and ACT so BOTH stay warm through the
    projection phase, and nothing off-PE sits on the score->AV path.
  - V chunk buffers ping-pong; the last V block's PSUM tile is copied
    TWICE (into this chunk's tile and the next chunk's halo slot) on the
    warm ACT engine, so there is no cross-chunk halo copy on a cold
    engine. Score lhsT tiles for the halo block come straight from the
    previous chunk's x tile.
"""

import sys

if "/opt/trn_rl_repo" not in sys.path:
    sys.path.insert(0, "/opt/trn_rl_repo")

import numpy as np
from contextlib import ExitStack

import os

import concourse.bass as bass
import concourse.mybir as mybir
import concourse.tile as tile
from concourse.bass_utils import run_bass_kernel_spmd
from concourse.vector_clock import ScopedClock

_ORIG_RUN = run_bass_kernel_spmd

# ---------------------------------------------------------------------------
# Problem constants (hardcoded per spec)
B, T, D = 4, 4096, 1024
SPAN = 128
NCORES = 8
TOKQ = T // 2           # 2048 query tokens per core
HALO = SPAN             # 128
CH = 256                # query chunk size (2 blocks = 1 block-pair)
NCH = TOKQ // CH        # 8 chunks
KVW = CH + HALO         # 384 KV tokens visible per chunk
DT = D // 128           # 8 d-tiles
SCALE = 1.0 / 32.0      # 1/sqrt(D)
EXP_BIAS = -4.0         # folded into exp; cancels in softmax, avoids f16 inf

F32 = mybir.dt.float32
F16 = mybir.dt.float16

# ---------------------------------------------------------------------------
# Walrus in this toolchain caps semaphore waits per instruction; Tile's
# kernel-tail Drain can exceed it. Chunk excess waits onto extra drains.
_MAX_WAITS = 1


def _patched_drain_and_barrier(self, tick_clock, wait_clock):
    nc = self.nc
    drain_inst = nc.sync.drain()
    wait_clock.add_sem_waits(
        drain_inst.ins, ScopedClock({None: tick_clock.global_clock})
    )
    si = drain_inst.ins.sync_info
    if si is not None and len(si.on_wait) > _MAX_WAITS:
        waits = list(si.on_wait)
        si.on_wait[:] = waits[:_MAX_WAITS]
        rest = waits[_MAX_WAITS:]
        while rest:
            extra = nc.sync.drain(fusable=False)
            extra.ins.sync_info = mybir.SyncInfo(
                on_wait=rest[:_MAX_WAITS], on_update=[]
            )
            rest = rest[_MAX_WAITS:]
    nc.all_engine_barrier()
    assert self.sems is not None
    popped = nc._tile_sem_poison_stack.pop()
    assert popped is self._sem_poison
    nc.clear_and_free_semaphores(list(self.sems.allocated().values()))
    nc.all_engine_barrier()


def _install_drain_patch():
    if getattr(tile.TileContext, "_drain_patch_installed", False):
        return
    tile.TileContext._drain_and_barrier = _patched_drain_and_barrier
    tile.TileContext._drain_patch_installed = True


def _split_multi_waits(nc, max_waits=_MAX_WAITS):
    """Walrus here supports one semaphore wait per instruction; hoist excess
    waits onto same-engine NoOps inserted immediately before."""
    for fn in nc.m.functions:
        for bb in fn.blocks:
            insts = bb.instructions
            out = []
            changed = False
            for inst in insts:
                si = getattr(inst, "sync_info", None)
                waits = list(si.on_wait) if si is not None else []
                if len(waits) > max_waits:
                    changed = True
                    for w in waits[:-max_waits]:
                        out.append(mybir.InstNoOp(
                            name=nc.get_next_instruction_name(),
                            sync_info=mybir.SyncInfo(on_wait=[w], on_update=[]),
                            bass_nofuse=True,
                            engine=inst.engine,
                        ))
                    si.on_wait[:] = waits[-max_waits:]
                out.append(inst)
            if changed:
                bb.instructions = out


# ---------------------------------------------------------------------------
def _build_graph():
    """Build the per-core Bass graph (SPMD: identical on all 8 cores)."""
    _install_drain_patch()
    nc = bass.Bass()

    xT = nc.declare_dram_parameter("xT", [D, HALO + TOKQ], F16, isOutput=False)
    aT = nc.declare_dram_parameter("aT", [D, D], F16, isOutput=False)
    wvT = nc.declare_dram_parameter("wvT", [D, D], F16, isOutput=False)
    w8 = nc.declare_dram_parameter("w8", [128, DT], F32, isOutput=False)
    bvb = nc.declare_dram_parameter("bvb", [128, D], F32, isOutput=False)
    maskp = nc.declare_dram_parameter("maskp", [128, 2 * CH], F16, isOutput=False)
    onesp = nc.declare_dram_parameter("onesp", [128, 8], F16, isOutput=False)
    identp = nc.declare_dram_parameter("identp", [128, 128], F16, isOutput=False)
    out = nc.declare_dram_parameter("out", [TOKQ, D], F16, isOutput=True)

    with ExitStack() as ctx:
        tc = ctx.enter_context(tile.TileContext(nc))
        consts = ctx.enter_context(tc.tile_pool(name="consts", bufs=1))
        persist = ctx.enter_context(tc.tile_pool(name="persist", bufs=1))
        xc0p = ctx.enter_context(tc.tile_pool(name="xc0p", bufs=1))
        xcp = ctx.enter_context(tc.tile_pool(name="xcp", bufs=3))
        work = ctx.enter_context(tc.tile_pool(name="work", bufs=3))
        outp = ctx.enter_context(tc.tile_pool(name="outp", bufs=2))
        psp = ctx.enter_context(tc.tile_pool(name="psp", bufs=2, space="PSUM"))
        pss = ctx.enter_context(tc.tile_pool(name="pss", bufs=2, space="PSUM"))
        pso = ctx.enter_context(tc.tile_pool(name="pso", bufs=2, space="PSUM"))

        # --- resident weights + constants -------------------------------
        # DMA order = first-need order: tiny constants, then chunk-0 x
        # (gates everything), then A in e-halves (Z chains e0-3 need only
        # the first half-columns of every A tile), then Wv in eh-halves
        # (V chains run eh-outer), then bv (first needed at the first
        # normalize, ~25us in).
        # PE warmup: wide dummy matmuls run while the initial DMAs stream
        # in, so the HAM clock gate is at 8/8 (2.4 GHz) by the time real
        # matmuls start (it otherwise stays at 1.2 GHz through chunk 0).
        warm = consts.tile([128, 256], F16, tag="warm")
        nc.vector.memset(warm, 0.0)
        ebias = consts.tile([128, 1], F32, tag="ebias")
        nc.vector.memset(ebias, EXP_BIAS)
        kw_dve = consts.tile([128, 1], F32, tag="kw_dve")
        kw_act = consts.tile([128, 1], F32, tag="kw_act")
        nc.scalar.copy(kw_act, ebias)
        ps_w = pss.tile([128, CH + 16], F32, tag="sT")
        for _ in range(40):
            nc.tensor.matmul(
                ps_w[:, 0:CH], warm[:, 0:128], warm, start=True, stop=True
            )

        w_sb = consts.tile([128, DT], F32, tag="w8")
        nc.sync.dma_start(out=w_sb, in_=w8[:, :])
        mask_sb = consts.tile([128, 2 * CH], F16, tag="mask")
        nc.sync.dma_start(out=mask_sb, in_=maskp[:, :])
        ones_sb = consts.tile([128, 8], F16, tag="ones")
        nc.sync.dma_start(out=ones_sb, in_=onesp[:, :])
        ident_sb = consts.tile([128, 128], F16, tag="ident")
        nc.sync.dma_start(out=ident_sb, in_=identp[:, :])

        # chunk-0 x: own columns first (they gate the first Z chains); the
        # halo columns are only needed by the V/score stages ~8us later
        xc0 = xc0p.tile([128, DT, KVW], F16, tag="xc0")
        for d in range(DT):
            nc.sync.dma_start(
                out=xc0[:, d, HALO:KVW], in_=xT[d * 128:(d + 1) * 128, HALO:KVW]
            )
        a_sb = [consts.tile([128, D], F16, tag=f"a{d}", name=f"a{d}")
                for d in range(DT)]
        wv_sb = [consts.tile([128, D], F16, tag=f"wv{d}", name=f"wv{d}")
                 for d in range(DT)]
        # A lands in e-column groups in exactly the order the chunk-0 Z
        # chains consume them, so the first chain starts ~3us earlier
        for gi, (c0, c1) in enumerate([(0, 256), (256, 512), (512, 1024)]):
            for d in range(DT):
                nc.sync.dma_start(
                    out=a_sb[d][:, c0:c1],
                    in_=aT[d * 128:(d + 1) * 128, c0:c1],
                )
            # keep-warm tap: fires when this column group lands, so DVE
            # never sits cold for >~5us during the startup stream
            nc.vector.tensor_copy(kw_dve, a_sb[DT - 1][:, c0:c0 + 1])
            if gi == 0:
                for d in range(DT):
                    nc.sync.dma_start(
                        out=xc0[:, d, 0:HALO],
                        in_=xT[d * 128:(d + 1) * 128, 0:HALO],
                    )
        for eh in range(2):
            for d in range(DT):
                nc.sync.dma_start(
                    out=wv_sb[d][:, eh * 512:(eh + 1) * 512],
                    in_=wvT[d * 128:(d + 1) * 128, eh * 512:(eh + 1) * 512],
                )
            nc.scalar.copy(kw_act, wv_sb[DT - 1][:, eh * 512:eh * 512 + 1])
        bv_sb = consts.tile([128, D], F32, tag="bv")
        nc.sync.dma_start(out=bv_sb, in_=bvb[:, :])

        # --- persistent per-chunk state (ping-pong so chunk c+1's
        # projections never write-after-read-stall on chunk c's attention) --
        # ZT: Z^T, d'-tile-major [128, d'_tile, q_col], q_col in [0, 256)
        # V: token-tile-major [128, tok_tile(3), e]; tile 0 = halo block
        ZT2 = [persist.tile([128, DT, CH], F16, tag=f"ZT{i}", name=f"ZT{i}")
               for i in range(2)]
        V2 = [persist.tile([128, KVW // 128, D], F16, tag=f"V{i}", name=f"V{i}")
              for i in range(2)]

        xc_hist = [None] * NCH

        for c in range(NCH):
            ZT, V = ZT2[c % 2], V2[c % 2]
            # ---- x chunk DMA (d-major) ---------------------------------
            if c == 0:
                # xc0 (incl. halo: xT cols [0, 384)) was DMA'd up front
                xc = xc0
                own0 = HALO          # xc col of first own token
                kv_t0 = 0            # first KV token-tile to project
            else:
                xc = xcp.tile([128, DT, CH], F16, tag="xc")
                lo = HALO + c * CH
                for d in range(DT):
                    nc.sync.dma_start(
                        out=xc[:, d, :], in_=xT[d * 128:(d + 1) * 128, lo:lo + CH]
                    )
                own0 = 0
                kv_t0 = 1
            xc_hist[c] = xc

            # ---- Z^T projection (the fused Q/K side) -------------------
            # PSUM->SBUF bias-copies alternate DVE/ACT to keep both warm
            for e in range(DT):
                ps = psp.tile([128, 512], F32, tag="proj")
                for d in range(DT):
                    nc.tensor.matmul(
                        ps[:, 0:CH],
                        a_sb[d][:, e * 128:(e + 1) * 128],
                        xc[:, d, own0:own0 + CH],
                        start=(d == 0),
                        stop=(d == DT - 1),
                    )
                if e % 2 == 0:
                    nc.vector.tensor_scalar_add(
                        ZT[:, e, :], ps[:, 0:CH], w_sb[:, e:e + 1]
                    )
                else:
                    nc.scalar.add(ZT[:, e, :], ps[:, 0:CH], w_sb[:, e:e + 1])
                if c == 0 and e < 6:
                    # chunk 0 is DMA-latency-bound: pad the PE with dummy
                    # matmuls between chains so it stays busy (and warm)
                    # while x/A/Wv tiles stream in.
                    for _ in range(3):
                        nc.tensor.matmul(
                            ps_w[:, 0:CH], warm[:, 0:128], warm,
                            start=True, stop=True,
                        )

            # ---- V projection (no bias: bv folds into the final
            # normalize; eh-outer so chunk 0 consumes Wv's first column
            # half as soon as it lands) ----------------------------------
            for eh in range(2):
                for t in range(kv_t0, KVW // 128):
                    xcol = (t - kv_t0) * 128
                    ps = psp.tile([128, 512], F32, tag="proj")
                    for d in range(DT):
                        nc.tensor.matmul(
                            ps,
                            xc[:, d, xcol:xcol + 128],
                            wv_sb[d][:, eh * 512:(eh + 1) * 512],
                            start=(d == 0),
                            stop=(d == DT - 1),
                        )
                    dst = V[:, t, eh * 512:(eh + 1) * 512]
                    if t == 1:
                        nc.vector.tensor_copy(dst, ps)
                    else:
                        nc.scalar.copy(dst, ps)
                    if t == 2 and c < NCH - 1:
                        # the last V block is also next chunk's halo:
                        # write it into both tiles straight from PSUM
                        nc.scalar.copy(
                            V2[(c + 1) % 2][:, 0, eh * 512:(eh + 1) * 512], ps
                        )
                    if c == 0 and not (eh == 1 and t == 2):
                        # elastic PE filler: on slow-DMA runs the V chains
                        # straggle behind the Wv stream; a single PE gap
                        # here drops the HAM clock gate to 1.2GHz for the
                        # next ~15us of real work, which costs far more
                        # than these dummy rows do on fast runs.
                        for _ in range(3):
                            nc.tensor.matmul(
                                ps_w[:, 0:CH], warm[:, 0:128], warm,
                                start=True, stop=True,
                            )

            # ---- attention, one 128-query half at a time ---------------
            # sT[k, ri*128 + q] = sum_d xT[d, k]*ZT[d, q]; k-blocks are the
            # only two the window touches (r = half: prev block, half+1:
            # own block). cols [256:264) hold the softmax denominator.
            for half in range(2):
                q0 = half * 128
                # chunk-0 half A: the halo block mask also kills the
                # sequence-start padding (all -1e4 on h=0 cores, via data)
                mslot = 0 if (c == 0 and half == 0) else 1
                ps_s = pss.tile([128, CH + 16], F32, tag="sT")
                for ri in range(2):
                    r = half + ri      # 0 = halo block, 1/2 = own blocks
                    # seed the accumulation with the additive mask
                    # (identity lhsT), keeping masking on the PE
                    mcol = mslot * CH + ri * 128
                    nc.tensor.matmul(
                        ps_s[:, ri * 128:(ri + 1) * 128],
                        ident_sb,
                        mask_sb[:, mcol:mcol + 128],
                        start=True,
                        stop=False,
                    )
                    for d in range(DT):
                        if c == 0:
                            lhsT = xc0[:, d, r * 128:(r + 1) * 128]
                        elif r == 0:
                            pcol = KVW - 128 if c == 1 else CH - 128
                            lhsT = xc_hist[c - 1][:, d, pcol:pcol + 128]
                        else:
                            lhsT = xc[:, d, (r - 1) * 128:r * 128]
                        nc.tensor.matmul(
                            ps_s[:, ri * 128:(ri + 1) * 128],
                            lhsT,
                            ZT[:, d, q0:q0 + 128],
                            start=False,
                            stop=(d == DT - 1),
                        )
                # wT = exp(sT - 4)   (the -4 cancels in softmax and keeps
                # everything comfortably inside f16 range; masked entries
                # are exp(-1e4) = 0)
                wT = work.tile([128, CH], F16, tag="wT")
                nc.scalar.activation(
                    wT, ps_s[:, 0:CH], mybir.ActivationFunctionType.Exp,
                    bias=ebias[:, 0:1],
                )

                # denominator (ones-matmul into the score PSUM tail) + AV
                ps_o = pso.tile([128, 1024], F32, tag="o")
                for ri in range(2):
                    lhsT = wT[:, ri * 128:(ri + 1) * 128]
                    for eh in range(2):
                        nc.tensor.matmul(
                            ps_o[:, eh * 512:(eh + 1) * 512],
                            lhsT,
                            V[:, half + ri, eh * 512:(eh + 1) * 512],
                            start=(ri == 0),
                            stop=(ri == 1),
                        )
                    nc.tensor.matmul(
                        ps_s[:, CH:CH + 8],
                        lhsT,
                        ones_sb,
                        start=(ri == 0),
                        stop=(ri == 1),
                    )
                recip = outp.tile([128, 1], F32, tag="recip")
                nc.vector.reciprocal(recip, ps_s[:, CH:CH + 1])
                # o = ps_o * (1/den) + bv, fused, f16 out
                o_sb = outp.tile([128, D], F16, tag="o_sb")
                nc.vector.scalar_tensor_tensor(
                    o_sb[:, 0:512], ps_o[:, 0:512], recip, bv_sb[:, 0:512],
                    mybir.AluOpType.mult, mybir.AluOpType.add,
                )
                nc.vector.scalar_tensor_tensor(
                    o_sb[:, 512:1024], ps_o[:, 512:1024], recip,
                    bv_sb[:, 512:1024],
                    mybir.AluOpType.mult, mybir.AluOpType.add,
                )
                row0 = c * CH + half * 128
                nc.sync.dma_start(out=out[row0:row0 + 128, :], in_=o_sb)

        # Trailer dummies: keep the PE (and so the HAM clock gate) busy
        # through the final output DMA / drain window, so the fixed NRT
        # semaphore-reset epilogue (~8us of sequencer ops) starts at
        # 2.4GHz instead of dropping to the 1.2GHz idle clock.
        for _ in range(10):
            nc.tensor.matmul(
                ps_w[:, 0:CH], warm[:, 0:128], warm, start=True, stop=True
            )

    _split_multi_waits(nc)
    return nc


_GRAPH = None


def _get_graph():
    global _GRAPH
    if _GRAPH is None:
        _GRAPH = _build_graph()
    return _GRAPH


# Cached jitted executable: run_bass_via_pjrt rebuilds its jit closure per
# call (a full retrace each time); replicate its multi-core path once and
# reuse it so repeated kernel() calls cost only the device execution.
_RUNNER = None


def _fast_run(nc, in_maps):
    global _RUNNER
    import jax
    from jax.experimental.shard_map import shard_map
    from jax.sharding import Mesh, PartitionSpec
    from concourse import bass2jax

    n_cores = len(in_maps)
    if _RUNNER is None:
        bass2jax.install_neuronx_cc_hook()
        partition_name = (
            nc.partition_id_tensor.name if nc.partition_id_tensor else None
        )
        in_names, out_names, out_avals = [], [], []
        for alloc in nc.m.functions[0].allocations:
            if not isinstance(alloc, mybir.MemoryLocationSet):
                continue
            name = alloc.memorylocations[0].name
            if alloc.kind == "ExternalInput":
                if name != partition_name:
                    in_names.append(name)
            elif alloc.kind == "ExternalOutput":
                out_names.append(name)
                out_avals.append(jax.core.ShapedArray(
                    tuple(alloc.tensor_shape), mybir.dt.np(alloc.dtype)))
        n_params = len(in_names)
        all_names = in_names + out_names
        if partition_name is not None:
            all_names.append(partition_name)
        donate = tuple(range(n_params, n_params + len(out_names)))

        def _body(*args):
            operands = list(args)
            if partition_name is not None:
                operands.append(bass2jax.partition_id_tensor())
            return tuple(bass2jax._bass_exec_p.bind(
                *operands,
                out_avals=tuple(out_avals),
                in_names=tuple(all_names),
                out_names=tuple(out_names),
                lowering_input_output_aliases=(),
                sim_require_finite=True,
                sim_require_nnan=True,
                nc=nc,
            ))

        devices = jax.devices()[:n_cores]
        mesh = Mesh(np.asarray(devices), ("core",))
        nio = n_params + len(out_names)
        sharded = jax.jit(
            shard_map(
                _body, mesh=mesh,
                in_specs=(PartitionSpec("core"),) * nio,
                out_specs=(PartitionSpec("core"),) * len(out_names),
                check_rep=False,
            ),
            donate_argnums=donate,
            keep_unused=True,
        )
        # allocate the donated output buffers on-device (no host transfer)
        import jax.numpy as jnp
        from jax.sharding import NamedSharding
        shardings = tuple(
            NamedSharding(mesh, PartitionSpec("core")) for _ in out_avals
        )
        zeros_jit = jax.jit(
            lambda: tuple(
                jnp.zeros((n_cores * av.shape[0], *av.shape[1:]), av.dtype)
                for av in out_avals
            ),
            out_shardings=shardings,
        )
        _RUNNER = (sharded, zeros_jit, in_names, out_names, out_avals)

    sharded, zeros_jit, in_names, out_names, out_avals = _RUNNER
    concat_in = [
        np.concatenate([np.asarray(m[name]) for m in in_maps], axis=0)
        for name in in_names
    ]
    out_arrs = sharded(*concat_in, *zeros_jit())
    return [
        {
            name: np.asarray(out_arrs[i]).reshape(
                n_cores, *out_avals[i].shape)[c]
            for i, name in enumerate(out_names)
        }
        for c in range(n_cores)
    ]


# ---------------------------------------------------------------------------
def _make_masks():
    """Additive {0, -1e4} masks in [k_partition, ri*128 + q_free] layout.

    slot 0: [mP0 | mC]  — chunk-0 half A (mP0 is all -1e4 on h=0 cores)
    slot 1: [mP  | mC]  — everything else
    mP: previous-block keys, in-window iff k > q; mC: own block, k <= q.
    """
    NEG = np.float16(-10000.0)
    kp = np.arange(128)[:, None]
    qf = np.arange(128)[None, :]
    mP = np.where(kp > qf, np.float16(0), NEG)
    mC = np.where(kp <= qf, np.float16(0), NEG)
    m1 = np.concatenate([mP, mC], axis=1)
    m0_h0 = np.concatenate([np.full_like(mP, NEG), mC], axis=1)
    return np.concatenate([m0_h0, m1], axis=1), np.concatenate([m1, m1], axis=1)


def kernel(x, Wq, bq, Wk, bk, Wv, bv, span):
    x = np.asarray(x)
    span_i = int(np.asarray(span))
    assert span_i == SPAN, f"kernel hardcodes span={SPAN}, got {span_i}"
    assert x.shape == (B, T, D)

    nc = _get_graph()

    Wq = np.asarray(Wq, dtype=np.float32)
    Wk = np.asarray(Wk, dtype=np.float32)
    bq_f = np.asarray(bq, dtype=np.float32)
    # A = scale * Wq^T Wk ; w = scale * bq Wk  (host-side fusion: the whole
    # K projection and both per-query score terms vanish — softmax is
    # invariant to per-query constants)
    A = (Wq.T @ Wk) * np.float32(SCALE)
    wrow = (bq_f @ Wk) * np.float32(SCALE)
    aT = np.ascontiguousarray(A).astype(np.float16)
    wvT = np.ascontiguousarray(np.asarray(Wv).T).astype(np.float16)
    w8 = np.ascontiguousarray(wrow.reshape(DT, 128).T).astype(np.float32, copy=False)
    bvb = np.ascontiguousarray(np.broadcast_to(np.asarray(bv), (128, D))).astype(np.float32, copy=False)
    m_h0, m_h1 = _make_masks()
    ones = np.ones((128, 8), np.float16)
    ident = np.eye(128, dtype=np.float16)

    in_maps = []
    for core in range(NCORES):
        b, h = divmod(core, 2)
        lo = h * TOKQ - HALO
        hi = (h + 1) * TOKQ
        xs = np.zeros((HALO + TOKQ, D), np.float32)
        if lo < 0:
            xs[HALO:] = x[b, 0:hi]
        else:
            xs[:] = x[b, lo:hi]
        xT = np.ascontiguousarray(xs.T).astype(np.float16)
        in_maps.append({
            "xT": xT, "aT": aT, "wvT": wvT, "w8": w8, "bvb": bvb,
            "maskp": (m_h0 if h == 0 else m_h1), "onesp": ones,
            "identp": ident,
        })

    if run_bass_kernel_spmd is _ORIG_RUN and not os.environ.get("BASS_TRACE"):
        results = _fast_run(nc, in_maps)
    else:
        # a harness monkeypatched run_bass_kernel_spmd (e.g. for tracing)
        results = run_bass_kernel_spmd(
            nc, in_maps, core_ids=list(range(NCORES))
        ).results

    out = np.empty((B, T, D), np.float32)
    for core in range(NCORES):
        b, h = divmod(core, 2)
        out[b, h * TOKQ:(h + 1) * TOKQ] = results[core]["out"]
    return out


# revision 24
# speedup vs baseline: 1.0408x; 1.0408x over previous
"""Trainium2 kernel for AdaptiveAttention (QKV projection + causal
sliding-window attention, span=128) on 8 NeuronCores.

Sharding: sequence-parallel with a 1-block halo — 8 shards of
(batch b, sequence half h): each core owns 2048 query tokens and receives
128 halo tokens of x so the previous block's K/V context is local.
No collectives are needed (window attention is local).

Algorithmic fusion (the big win vs a direct QKV translation): softmax is
invariant to adding a per-query constant to the scores, so

    S = (x Wq^T + bq) (x Wk^T + bk)^T
      ~ x (Wq^T Wk) x^T + (bq Wk) x^T        (per-query terms dropped)

With A = scale * Wq^T Wk and w = scale * bq Wk precomputed on the HOST,
the whole K projection disappears from the device: the kernel computes
Z^T = A^T-contract(x) + w once per query chunk (exactly the cost of the
old Q projection) and scores contract Z directly against raw x tiles
(which are already resident in d-major layout for the projections).
Similarly bv is not added to V: sum_j softmax_ij (v_j + bv) = (...) + bv,
so bv is folded into the final normalize (one fused DVE op).

Per-core layout strategy:
  - All matmuls run in f16 (inputs rounded on host; the 2^-11 rounding is
    comparable to the f32r PE path's own rounding) at full PE rate.
  - x is passed pre-transposed (d-major) so the Z projection, V projection
    and the score matmuls all use resident tiles with no on-chip
    transposes:
      Z^T[d',q] = sum_d A[d,d'] * xT[d,q]     (lhsT=A tile,  rhs=xT)
      V[t,e]    = sum_d xT[d,t] * WvT[d,e]    (lhsT=xT tile, rhs=WvT)
      sT[k,q]   = sum_d xT[d,k] * Z^T[d,q]    (lhsT=xT tile, rhs=Z^T)
  - Scores are computed per 128-query half as two [128,128] blocks (only
    the blocks the window can touch). The causal/window mask is ADDITIVE
    (-1e4) and applied by the PE itself: each score accumulation chain is
    SEEDED with an identity-lhsT matmul of the mask tile (128 rows, ~2%
    of chunk PE time), so no non-PE engine sits between the score chains
    and the exp. exp runs on ACT straight from PSUM with a folded -4
    bias (cancels in softmax, keeps everything in f16 range), then
      o[q,e] = sum_k wT[k,q] * V[k,e]
    with the softmax denominator from a ones-column matmul into the score
    PSUM tile's tail; the final normalize is a per-partition scalar
    multiply fused with the +bv add and the PSUM->SBUF f16 cast.
  - Engine p-state discipline (the big scheduling lesson: an engine idle
    for >~5us drops to a low p-state and its next op runs ~10x slow, and
    any PE gap also resets the PE clock to 1.2GHz for ~3us): PSUM->SBUF
    copies alternate between DVE and ACT so BOTH stay warm through the
    projection phase, and nothing off-PE sits on the score->AV path.
  - V chunk buffers ping-pong; the last V block's PSUM tile is copied
    TWICE (into this chunk's tile and the next chunk's halo slot) on the
    warm ACT engine, so there is no cross-chunk halo copy on a cold
    engine. Score lhsT tiles for the halo block come straight from the
    previous chunk's x tile.
"""

import sys

if "/opt/trn_rl_repo" not in sys.path:
    sys.path.insert(0, "/opt/trn_rl_repo")

import numpy as np
from contextlib import ExitStack

import os

import concourse.bass as bass
import concourse.mybir as mybir
import concourse.tile as tile
from concourse.bass_utils import run_bass_kernel_spmd
from concourse.vector_clock import ScopedClock

_ORIG_RUN = run_bass_kernel_spmd

# ---------------------------------------------------------------------------
# Problem constants (hardcoded per spec)
B, T, D = 4, 4096, 1024
SPAN = 128
NCORES = 8
TOKQ = T // 2           # 2048 query tokens per core
HALO = SPAN             # 128
CH = 256                # query chunk size (2 blocks = 1 block-pair)
NCH = TOKQ // CH        # 8 chunks
KVW = CH + HALO         # 384 KV tokens visible per chunk
DT = D // 128           # 8 d-tiles
SCALE = 1.0 / 32.0      # 1/sqrt(D)
EXP_BIAS = -4.0         # folded into exp; cancels in softmax, avoids f16 inf

F32 = mybir.dt.float32
F16 = mybir.dt.float16

# ---------------------------------------------------------------------------
# Walrus in this toolchain caps semaphore waits per instruction; Tile's
# kernel-tail Drain can exceed it. Chunk excess waits onto extra drains.
_MAX_WAITS = 1


def _patched_drain_and_barrier(self, tick_clock, wait_clock):
    nc = self.nc
    drain_inst = nc.sync.drain()
    wait_clock.add_sem_waits(
        drain_inst.ins, ScopedClock({None: tick_clock.global_clock})
    )
    si = drain_inst.ins.sync_info
    if si is not None and len(si.on_wait) > _MAX_WAITS:
        waits = list(si.on_wait)
        si.on_wait[:] = waits[:_MAX_WAITS]
        rest = waits[_MAX_WAITS:]
        while rest:
            extra = nc.sync.drain(fusable=False)
            extra.ins.sync_info = mybir.SyncInfo(
                on_wait=rest[:_MAX_WAITS], on_update=[]
            )
            rest = rest[_MAX_WAITS:]
    nc.all_engine_barrier()
    assert self.sems is not None
    popped = nc._tile_sem_poison_stack.pop()
    assert popped is self._sem_poison
    nc.clear_and_free_semaphores(list(self.sems.allocated().values()))
    nc.all_engine_barrier()


def _install_drain_patch():
    if getattr(tile.TileContext, "_drain_patch_installed", False):
        return
    tile.TileContext._drain_and_barrier = _patched_drain_and_barrier
    tile.TileContext._drain_patch_installed = True


def _split_multi_waits(nc, max_waits=_MAX_WAITS):
    """Walrus here supports one semaphore wait per instruction; hoist excess
    waits onto same-engine NoOps inserted immediately before."""
    for fn in nc.m.functions:
        for bb in fn.blocks:
            insts = bb.instructions
            out = []
            changed = False
            for inst in insts:
                si = getattr(inst, "sync_info", None)
                waits = list(si.on_wait) if si is not None else []
                if len(waits) > max_waits:
                    changed = True
                    for w in waits[:-max_waits]:
                        out.append(mybir.InstNoOp(
                            name=nc.get_next_instruction_name(),
                            sync_info=mybir.SyncInfo(on_wait=[w], on_update=[]),
                            bass_nofuse=True,
                            engine=inst.engine,
                        ))
                    si.on_wait[:] = waits[-max_waits:]
                out.append(inst)
            if changed:
                bb.instructions = out


# ---------------------------------------------------------------------------
def _build_graph():
    """Build the per-core Bass graph (SPMD: identical on all 8 cores)."""
    _install_drain_patch()
    nc = bass.Bass()

    xT = nc.declare_dram_parameter("xT", [D, HALO + TOKQ], F16, isOutput=False)
    aT = nc.declare_dram_parameter("aT", [D, D], F16, isOutput=False)
    wvT = nc.declare_dram_parameter("wvT", [D, D], F16, isOutput=False)
    w8 = nc.declare_dram_parameter("w8", [128, DT], F32, isOutput=False)
    bvb = nc.declare_dram_parameter("bvb", [128, D], F32, isOutput=False)
    maskp = nc.declare_dram_parameter("maskp", [128, 2 * CH], F16, isOutput=False)
    onesp = nc.declare_dram_parameter("onesp", [128, 8], F16, isOutput=False)
    identp = nc.declare_dram_parameter("identp", [128, 128], F16, isOutput=False)
    out = nc.declare_dram_parameter("out", [TOKQ, D], F16, isOutput=True)

    with ExitStack() as ctx:
        tc = ctx.enter_context(tile.TileContext(nc))
        consts = ctx.enter_context(tc.tile_pool(name="consts", bufs=1))
        persist = ctx.enter_context(tc.tile_pool(name="persist", bufs=1))
        xc0p = ctx.enter_context(tc.tile_pool(name="xc0p", bufs=1))
        xcp = ctx.enter_context(tc.tile_pool(name="xcp", bufs=3))
        work = ctx.enter_context(tc.tile_pool(name="work", bufs=3))
        outp = ctx.enter_context(tc.tile_pool(name="outp", bufs=2))
        psp = ctx.enter_context(tc.tile_pool(name="psp", bufs=2, space="PSUM"))
        pss = ctx.enter_context(tc.tile_pool(name="pss", bufs=2, space="PSUM"))
        pso = ctx.enter_context(tc.tile_pool(name="pso", bufs=2, space="PSUM"))

        # --- resident weights + constants -------------------------------
        # DMA order = first-need order: tiny constants, then chunk-0 x
        # (gates everything), then A in e-halves (Z chains e0-3 need only
        # the first half-columns of every A tile), then Wv in eh-halves
        # (V chains run eh-outer), then bv (first needed at the first
        # normalize, ~25us in).
        # PE warmup: wide dummy matmuls run while the initial DMAs stream
        # in, so the HAM clock gate is at 8/8 (2.4 GHz) by the time real
        # matmuls start (it otherwise stays at 1.2 GHz through chunk 0).
        warm = consts.tile([128, 256], F16, tag="warm")
        nc.vector.memset(warm, 0.0)
        ebias = consts.tile([128, 1], F32, tag="ebias")
        nc.vector.memset(ebias, EXP_BIAS)
        kw_dve = consts.tile([128, 1], F32, tag="kw_dve")
        kw_act = consts.tile([128, 1], F32, tag="kw_act")
        nc.scalar.copy(kw_act, ebias)
        ps_w = pss.tile([128, CH + 16], F32, tag="sT")
        for _ in range(40):
            nc.tensor.matmul(
                ps_w[:, 0:CH], warm[:, 0:128], warm, start=True, stop=True
            )

        w_sb = consts.tile([128, DT], F32, tag="w8")
        nc.sync.dma_start(out=w_sb, in_=w8[:, :])
        mask_sb = consts.tile([128, 2 * CH], F16, tag="mask")
        nc.sync.dma_start(out=mask_sb, in_=maskp[:, :])
        ones_sb = consts.tile([128, 8], F16, tag="ones")
        nc.sync.dma_start(out=ones_sb, in_=onesp[:, :])
        ident_sb = consts.tile([128, 128], F16, tag="ident")
        nc.sync.dma_start(out=ident_sb, in_=identp[:, :])

        # chunk-0 x as whole-tile DMAs (768B bursts; finer column splits
        # measurably degrade the DMA stream rate)
        xc0 = xc0p.tile([128, DT, KVW], F16, tag="xc0")
        for d in range(DT):
            nc.sync.dma_start(
                out=xc0[:, d, :], in_=xT[d * 128:(d + 1) * 128, 0:KVW]
            )
        a_sb = [consts.tile([128, D], F16, tag=f"a{d}", name=f"a{d}")
                for d in range(DT)]
        wv_sb = [consts.tile([128, D], F16, tag=f"wv{d}", name=f"wv{d}")
                 for d in range(DT)]
        # A lands in two e-column half-groups (1KB bursts) in the order
        # the chunk-0 Z chains consume them, so chains e0-3 start early
        for eg in range(2):
            for d in range(DT):
                nc.sync.dma_start(
                    out=a_sb[d][:, eg * 512:(eg + 1) * 512],
                    in_=aT[d * 128:(d + 1) * 128, eg * 512:(eg + 1) * 512],
                )
            # keep-warm tap: fires when this column group lands, so DVE
            # never sits cold for >~5us during the startup stream
            nc.vector.tensor_copy(kw_dve, a_sb[DT - 1][:, eg * 512:eg * 512 + 1])
        for eh in range(2):
            for d in range(DT):
                nc.sync.dma_start(
                    out=wv_sb[d][:, eh * 512:(eh + 1) * 512],
                    in_=wvT[d * 128:(d + 1) * 128, eh * 512:(eh + 1) * 512],
                )
            nc.scalar.copy(kw_act, wv_sb[DT - 1][:, eh * 512:eh * 512 + 1])
        bv_sb = consts.tile([128, D], F32, tag="bv")
        nc.sync.dma_start(out=bv_sb, in_=bvb[:, :])

        # --- persistent per-chunk state (ping-pong so chunk c+1's
        # projections never write-after-read-stall on chunk c's attention) --
        # ZT: Z^T, d'-tile-major [128, d'_tile, q_col], q_col in [0, 256)
        # V: token-tile-major [128, tok_tile(3), e]; tile 0 = halo block
        ZT2 = [persist.tile([128, DT, CH], F16, tag=f"ZT{i}", name=f"ZT{i}")
               for i in range(2)]
        V2 = [persist.tile([128, KVW // 128, D], F16, tag=f"V{i}", name=f"V{i}")
              for i in range(2)]

        xc_hist = [None] * NCH

        for c in range(NCH):
            ZT, V = ZT2[c % 2], V2[c % 2]
            # ---- x chunk DMA (d-major) ---------------------------------
            if c == 0:
                # xc0 (incl. halo: xT cols [0, 384)) was DMA'd up front
                xc = xc0
                own0 = HALO          # xc col of first own token
                kv_t0 = 0            # first KV token-tile to project
            else:
                xc = xcp.tile([128, DT, CH], F16, tag="xc")
                lo = HALO + c * CH
                for d in range(DT):
                    nc.sync.dma_start(
                        out=xc[:, d, :], in_=xT[d * 128:(d + 1) * 128, lo:lo + CH]
                    )
                own0 = 0
                kv_t0 = 1
            xc_hist[c] = xc

            # ---- Z^T projection (the fused Q/K side) -------------------
            # PSUM->SBUF bias-copies alternate DVE/ACT to keep both warm
            for e in range(DT):
                ps = psp.tile([128, 512], F32, tag="proj")
                for d in range(DT):
                    nc.tensor.matmul(
                        ps[:, 0:CH],
                        a_sb[d][:, e * 128:(e + 1) * 128],
                        xc[:, d, own0:own0 + CH],
                        start=(d == 0),
                        stop=(d == DT - 1),
                    )
                if e % 2 == 0:
                    nc.vector.tensor_scalar_add(
                        ZT[:, e, :], ps[:, 0:CH], w_sb[:, e:e + 1]
                    )
                else:
                    nc.scalar.add(ZT[:, e, :], ps[:, 0:CH], w_sb[:, e:e + 1])
                if c == 0 and e < 6:
                    # chunk 0 is DMA-latency-bound: pad the PE with dummy
                    # matmuls between chains so it stays busy (and warm)
                    # while x/A/Wv tiles stream in.
                    for _ in range(3):
                        nc.tensor.matmul(
                            ps_w[:, 0:CH], warm[:, 0:128], warm,
                            start=True, stop=True,
                        )

            # ---- V projection (no bias: bv folds into the final
            # normalize; eh-outer so chunk 0 consumes Wv's first column
            # half as soon as it lands) ----------------------------------
            for eh in range(2):
                for t in range(kv_t0, KVW // 128):
                    xcol = (t - kv_t0) * 128
                    ps = psp.tile([128, 512], F32, tag="proj")
                    for d in range(DT):
                        nc.tensor.matmul(
                            ps,
                            xc[:, d, xcol:xcol + 128],
                            wv_sb[d][:, eh * 512:(eh + 1) * 512],
                            start=(d == 0),
                            stop=(d == DT - 1),
                        )
                    dst = V[:, t, eh * 512:(eh + 1) * 512]
                    if t == 1:
                        nc.vector.tensor_copy(dst, ps)
                    else:
                        nc.scalar.copy(dst, ps)
                    if t == 2 and c < NCH - 1:
                        # the last V block is also next chunk's halo:
                        # write it into both tiles straight from PSUM
                        nc.scalar.copy(
                            V2[(c + 1) % 2][:, 0, eh * 512:(eh + 1) * 512], ps
                        )
                    if c == 0 and not (eh == 1 and t == 2):
                        # elastic PE filler: on slow-DMA runs the V chains
                        # straggle behind the Wv stream; a single PE gap
                        # here drops the HAM clock gate to 1.2GHz for the
                        # next ~15us of real work, which costs far more
                        # than these dummy rows do on fast runs.
                        for _ in range(3):
                            nc.tensor.matmul(
                                ps_w[:, 0:CH], warm[:, 0:128], warm,
                                start=True, stop=True,
                            )

            # ---- attention, one 128-query half at a time ---------------
            # sT[k, ri*128 + q] = sum_d xT[d, k]*ZT[d, q]; k-blocks are the
            # only two the window touches (r = half: prev block, half+1:
            # own block). cols [256:264) hold the softmax denominator.
            for half in range(2):
                q0 = half * 128
                # chunk-0 half A: the halo block mask also kills the
                # sequence-start padding (all -1e4 on h=0 cores, via data)
                mslot = 0 if (c == 0 and half == 0) else 1
                ps_s = pss.tile([128, CH + 16], F32, tag="sT")
                for ri in range(2):
                    r = half + ri      # 0 = halo block, 1/2 = own blocks
                    # seed the accumulation with the additive mask
                    # (identity lhsT), keeping masking on the PE
                    mcol = mslot * CH + ri * 128
                    nc.tensor.matmul(
                        ps_s[:, ri * 128:(ri + 1) * 128],
                        ident_sb,
                        mask_sb[:, mcol:mcol + 128],
                        start=True,
                        stop=False,
                    )
                    for d in range(DT):
                        if c == 0:
                            lhsT = xc0[:, d, r * 128:(r + 1) * 128]
                        elif r == 0:
                            pcol = KVW - 128 if c == 1 else CH - 128
                            lhsT = xc_hist[c - 1][:, d, pcol:pcol + 128]
                        else:
                            lhsT = xc[:, d, (r - 1) * 128:r * 128]
                        nc.tensor.matmul(
                            ps_s[:, ri * 128:(ri + 1) * 128],
                            lhsT,
                            ZT[:, d, q0:q0 + 128],
                            start=False,
                            stop=(d == DT - 1),
                        )
                # wT = exp(sT - 4)   (the -4 cancels in softmax and keeps
                # everything comfortably inside f16 range; masked entries
                # are exp(-1e4) = 0)
                wT = work.tile([128, CH], F16, tag="wT")
                nc.scalar.activation(
                    wT, ps_s[:, 0:CH], mybir.ActivationFunctionType.Exp,
                    bias=ebias[:, 0:1],
                )

                # denominator (ones-matmul into the score PSUM tail) + AV
                ps_o = pso.tile([128, 1024], F32, tag="o")
                for ri in range(2):
                    lhsT = wT[:, ri * 128:(ri + 1) * 128]
                    for eh in range(2):
                        nc.tensor.matmul(
                            ps_o[:, eh * 512:(eh + 1) * 512],
                            lhsT,
                            V[:, half + ri, eh * 512:(eh + 1) * 512],
                            start=(ri == 0),
                            stop=(ri == 1),
                        )
                    nc.tensor.matmul(
                        ps_s[:, CH:CH + 8],
                        lhsT,
                        ones_sb,
                        start=(ri == 0),
                        stop=(ri == 1),
                    )
                recip = outp.tile([128, 1], F32, tag="recip")
                nc.vector.reciprocal(recip, ps_s[:, CH:CH + 1])
                # o = ps_o * (1/den) + bv, fused, f16 out
                o_sb = outp.tile([128, D], F16, tag="o_sb")
                nc.vector.scalar_tensor_tensor(
                    o_sb[:, 0:512], ps_o[:, 0:512], recip, bv_sb[:, 0:512],
                    mybir.AluOpType.mult, mybir.AluOpType.add,
                )
                nc.vector.scalar_tensor_tensor(
                    o_sb[:, 512:1024], ps_o[:, 512:1024], recip,
                    bv_sb[:, 512:1024],
                    mybir.AluOpType.mult, mybir.AluOpType.add,
                )
                row0 = c * CH + half * 128
                nc.sync.dma_start(out=out[row0:row0 + 128, :], in_=o_sb)

        # Trailer dummies: keep the PE (and so the HAM clock gate) busy
        # through the final normalize/output-DMA/drain window (~2us), so
        # the fixed NRT semaphore-reset epilogue (~8us of sequencer ops)
        # starts at 2.4GHz instead of dropping to the 1.2GHz idle clock.
        for _ in range(20):
            nc.tensor.matmul(
                ps_w[:, 0:CH], warm[:, 0:128], warm, start=True, stop=True
            )

    _split_multi_waits(nc)
    return nc


_GRAPH = None


def _get_graph():
    global _GRAPH
    if _GRAPH is None:
        _GRAPH = _build_graph()
    return _GRAPH


# Cached jitted executable: run_bass_via_pjrt rebuilds its jit closure per
# call (a full retrace each time); replicate its multi-core path once and
# reuse it so repeated kernel() calls cost only the device execution.
_RUNNER = None


def _fast_run(nc, in_maps):
    global _RUNNER
    import jax
    from jax.experimental.shard_map import shard_map
    from jax.sharding import Mesh, PartitionSpec
    from concourse import bass2jax

    n_cores = len(in_maps)
    if _RUNNER is None:
        bass2jax.install_neuronx_cc_hook()
        partition_name = (
            nc.partition_id_tensor.name if nc.partition_id_tensor else None
        )
        in_names, out_names, out_avals = [], [], []
        for alloc in nc.m.functions[0].allocations:
            if not isinstance(alloc, mybir.MemoryLocationSet):
                continue
            name = alloc.memorylocations[0].name
            if alloc.kind == "ExternalInput":
                if name != partition_name:
                    in_names.append(name)
            elif alloc.kind == "ExternalOutput":
                out_names.append(name)
                out_avals.append(jax.core.ShapedArray(
                    tuple(alloc.tensor_shape), mybir.dt.np(alloc.dtype)))
        n_params = len(in_names)
        all_names = in_names + out_names
        if partition_name is not None:
            all_names.append(partition_name)
        donate = tuple(range(n_params, n_params + len(out_names)))

        def _body(*args):
            operands = list(args)
            if partition_name is not None:
                operands.append(bass2jax.partition_id_tensor())
            return tuple(bass2jax._bass_exec_p.bind(
                *operands,
                out_avals=tuple(out_avals),
                in_names=tuple(all_names),
                out_names=tuple(out_names),
                lowering_input_output_aliases=(),
                sim_require_finite=True,
                sim_require_nnan=True,
                nc=nc,
            ))

        devices = jax.devices()[:n_cores]
        mesh = Mesh(np.asarray(devices), ("core",))
        nio = n_params + len(out_names)
        sharded = jax.jit(
            shard_map(
                _body, mesh=mesh,
                in_specs=(PartitionSpec("core"),) * nio,
                out_specs=(PartitionSpec("core"),) * len(out_names),
                check_rep=False,
            ),
            donate_argnums=donate,
            keep_unused=True,
        )
        # allocate the donated output buffers on-device (no host transfer)
        import jax.numpy as jnp
        from jax.sharding import NamedSharding
        shardings = tuple(
            NamedSharding(mesh, PartitionSpec("core")) for _ in out_avals
        )
        zeros_jit = jax.jit(
            lambda: tuple(
                jnp.zeros((n_cores * av.shape[0], *av.shape[1:]), av.dtype)
                for av in out_avals
            ),
            out_shardings=shardings,
        )
        _RUNNER = (sharded, zeros_jit, in_names, out_names, out_avals)

    sharded, zeros_jit, in_names, out_names, out_avals = _RUNNER
    concat_in = [
        np.concatenate([np.asarray(m[name]) for m in in_maps], axis=0)
        for name in in_names
    ]
    out_arrs = sharded(*concat_in, *zeros_jit())
    return [
        {
            name: np.asarray(out_arrs[i]).reshape(
                n_cores, *out_avals[i].shape)[c]
            for i, name in enumerate(out_names)
        }
        for c in range(n_cores)
    ]


# ---------------------------------------------------------------------------
def _make_masks():
    """Additive {0, -1e4} masks in [k_partition, ri*128 + q_free] layout.

    slot 0: [mP0 | mC]  — chunk-0 half A (mP0 is all -1e4 on h=0 cores)
    slot 1: [mP  | mC]  — everything else
    mP: previous-block keys, in-window iff k > q; mC: own block, k <= q.
    """
    NEG = np.float16(-10000.0)
    kp = np.arange(128)[:, None]
    qf = np.arange(128)[None, :]
    mP = np.where(kp > qf, np.float16(0), NEG)
    mC = np.where(kp <= qf, np.float16(0), NEG)
    m1 = np.concatenate([mP, mC], axis=1)
    m0_h0 = np.concatenate([np.full_like(mP, NEG), mC], axis=1)
    return np.concatenate([m0_h0, m1], axis=1), np.concatenate([m1, m1], axis=1)


def kernel(x, Wq, bq, Wk, bk, Wv, bv, span):
    x = np.asarray(x)
    span_i = int(np.asarray(span))
    assert span_i == SPAN, f"kernel hardcodes span={SPAN}, got {span_i}"
    assert x.shape == (B, T, D)

    nc = _get_graph()

    Wq = np.asarray(Wq, dtype=np.float32)
    Wk = np.asarray(Wk, dtype=np.float32)
    bq_f = np.asarray(bq, dtype=np.float32)
    # A = scale * Wq^T Wk ; w = scale * bq Wk  (host-side fusion: the whole
    # K projection and both per-query score terms vanish — softmax is
    # invariant to per-query constants)
    A = (Wq.T @ Wk) * np.float32(SCALE)
    wrow = (bq_f @ Wk) * np.float32(SCALE)
    aT = np.ascontiguousarray(A).astype(np.float16)
    wvT = np.ascontiguousarray(np.asarray(Wv).T).astype(np.float16)
    w8 = np.ascontiguousarray(wrow.reshape(DT, 128).T).astype(np.float32, copy=False)
    bvb = np.ascontiguousarray(np.broadcast_to(np.asarray(bv), (128, D))).astype(np.float32, copy=False)
    m_h0, m_h1 = _make_masks()
    ones = np.ones((128, 8), np.float16)
    ident = np.eye(128, dtype=np.float16)

    in_maps = []
    for core in range(NCORES):
        b, h = divmod(core, 2)
        lo = h * TOKQ - HALO
        hi = (h + 1) * TOKQ
        xs = np.zeros((HALO + TOKQ, D), np.float32)
        if lo < 0:
            xs[HALO:] = x[b, 0:hi]
        else:
            xs[:] = x[b, lo:hi]
        xT = np.ascontiguousarray(xs.T).astype(np.float16)
        in_maps.append({
            "xT": xT, "aT": aT, "wvT": wvT, "w8": w8, "bvb": bvb,
            "maskp": (m_h0 if h == 0 else m_h1), "onesp": ones,
            "identp": ident,
        })

    if run_bass_kernel_spmd is _ORIG_RUN and not os.environ.get("BASS_TRACE"):
        results = _fast_run(nc, in_maps)
    else:
        # a harness monkeypatched run_bass_kernel_spmd (e.g. for tracing)
        results = run_bass_kernel_spmd(
            nc, in_maps, core_ids=list(range(NCORES))
        ).results

    out = np.empty((B, T, D), np.float32)
    for core in range(NCORES):
        b, h = divmod(core, 2)
        out[b, h * TOKQ:(h + 1) * TOKQ] = results[core]["out"]
    return out
